# revision 13
# baseline (speedup 1.0000x reference)
"""Trainium2 Bass kernel for BaseGraphAttNet (graph attention, bs=8, N=2048, H=512).

Strategy (data-parallel over batch, one batch per NeuronCore, 8 cores):
  device, per core (batch b):
    phase A: V = feats_b @ fc_w.T                          (PE, bf16)
    phase B: e^T[j,i] = adj_b[i,j] * exp(leaky(q[i]+k[j])) (ACT Prelu+Exp for 9
             j-tiles; GPSIMD computes leaky for the other 7 to unload ACT)
    phase C: unnorm_out = e^T.T @ V, denom = ones.T @ e^T  (PE, bf16)
  host:
    transposes (adj^T, feats^T), q/k vectors (tiny rank-1 projections),
    final normalize + residual: out = unnorm_out / denom + fc_b + feats.
    (fc_b moves out of V because softmax rows sum to 1.)

Phase C is emitted j-major over a first wave of 6 PSUM-resident output groups so
the PE chases ACT/GPSIMD production with minimal head-of-line stalls; remaining
output tiles run dense after production.

Key numerics facts:
  - masked logits for non-edges are ~-1e9 -> exp == 0.0 in fp32, so
    e = adj * exp(leaky(q_i+k_j)) reproduces the reference row-softmax after
    division by the row sum.
  - q_i errors are common to softmax row i and cancel in the normalization, so
    q may be broadcast through a bf16 K=1 matmul; k stays exact fp32 (ACT bias).
"""

import os
import sys
from contextlib import ExitStack

import numpy as np

sys.path.insert(0, "/opt/trn_rl_repo")

import ml_dtypes

BS, N, H = 8, 2048, 512
NCORES = 8
PART = 128
NT = N // PART  # 16 node tiles (both i and j)
HC = H // PART  # 4 contraction chunks for phase A
NIC = N // H  # 4 i-chunks of 512 for the denominator rows
LEAKY = 0.01
GJ = 4  # j-tiles per adjacency DMA (1 MB fp8 transfers)
GO = 4  # i-tiles per output DMA (1 MB fp32 transfers)
WAVE0 = 7  # i-tile groups resident in PSUM during production chase

# j-tiles whose leaky-relu runs on GPSIMD — disabled: walrus rejects
# tensor ops on the Pool engine (NCC_IXCG966)
GPS_JS = set()

USE_PRELU = True  # Prelu(alpha)==LeakyReLU, same ACT table set as Exp

_PROGRAM_CACHE = {}


def _build_program():
    import concourse.bacc as bacc
    import concourse.mybir as mybir
    import concourse.tile as tile

    f32 = mybir.dt.float32
    bf16 = mybir.dt.bfloat16
    fp8 = mybir.dt.float8e4
    AF = mybir.ActivationFunctionType
    OP = mybir.AluOpType

    nc = bacc.Bacc()

    adjT = nc.declare_dram_parameter("adjT", [N, N], bf16, isOutput=False)
    featsT = nc.declare_dram_parameter("featsT", [H, N], bf16, isOutput=False)
    fcwT = nc.declare_dram_parameter("fcwT", [H, H], bf16, isOutput=False)
    qv = nc.declare_dram_parameter("qv", [1, N], bf16, isOutput=False)
    kv = nc.declare_dram_parameter("kv", [PART, NT], f32, isOutput=False)
    out = nc.declare_dram_parameter("out", [N, H], f32, isOutput=True)
    den = nc.declare_dram_parameter("den", [1, N], f32, isOutput=True)

    with tile.TileContext(nc) as tc, ExitStack() as ctx:
        const = ctx.enter_context(tc.tile_pool(name="const", bufs=1))
        vpool = ctx.enter_context(tc.tile_pool(name="vpool", bufs=1))
        apool = ctx.enter_context(tc.tile_pool(name="apool", bufs=2))
        opool = ctx.enter_context(tc.tile_pool(name="opool", bufs=2))

        # ---- small loads first (q broadcast gates the ACT pipeline) ----
        qrow_sb = const.tile([1, N], bf16)
        nc.sync.dma_start(out=qrow_sb, in_=qv[:])
        kc_sb = const.tile([PART, NT], f32)  # k[j] per-partition, j-tile per col
        nc.sync.dma_start(out=kc_sb, in_=kv[:])
        ones_row = const.tile([1, PART], bf16)
        nc.vector.memset(ones_row, 1.0)
        ones_col = const.tile([PART, 1], bf16)
        nc.vector.memset(ones_col, 1.0)

        fcwT_sb = const.tile([PART, HC, H], bf16)
        nc.sync.dma_start(
            out=fcwT_sb, in_=fcwT[:].rearrange("(c p) n -> p c n", p=PART)
        )
        featsT_sb = const.tile([PART, HC, N], bf16)
        nc.sync.dma_start(
            out=featsT_sb, in_=featsT[:].rearrange("(c p) i -> p c i", p=PART)
        )

        qb_sb = const.tile([PART, N], f32)
        V_sb = vpool.tile([PART, NT, H], bf16)
        with tc.tile_pool(name="psA", bufs=2, space="PSUM") as psA:
            # q broadcast via K=1 matmul: ones[1,128].T @ q_row[1,512] per chunk
            for ic in range(NIC):
                pq = psA.tile([PART, H], f32, tag="pa")
                nc.tensor.matmul(
                    pq,
                    lhsT=ones_row,
                    rhs=qrow_sb[:, ic * H : (ic + 1) * H],
                    start=True,
                    stop=True,
                )
                nc.vector.tensor_copy(out=qb_sb[:, ic * H : (ic + 1) * H], in_=pq)

            # ---- phase A: V = feats @ fc_w.T (bias folded to host), bf16 ----
            for t in range(NT):
                pa = psA.tile([PART, H], f32, tag="pa")
                for c in range(HC):
                    nc.tensor.matmul(
                        pa,
                        lhsT=featsT_sb[:, c, t * PART : (t + 1) * PART],
                        rhs=fcwT_sb[:, c, :],
                        start=(c == 0),
                        stop=(c == HC - 1),
                    )
                nc.vector.tensor_copy(out=V_sb[:, t, :], in_=pa)

        # ---- phases B + C interleaved, j-major ----
        epool = ctx.enter_context(tc.tile_pool(name="epool", bufs=1))
        work = ctx.enter_context(tc.tile_pool(name="work", bufs=2))
        gwork = ctx.enter_context(tc.tile_pool(name="gwork", bufs=1))
        e_tiles = [
            epool.tile([PART, N], bf16, tag=f"e{j}", name=f"e{j}")
            for j in range(NT)
        ]
        den_row = const.tile([1, N], f32)

        psC = ctx.enter_context(tc.tile_pool(name="psC", bufs=WAVE0, space="PSUM"))
        psD = ctx.enter_context(tc.tile_pool(name="psD", bufs=1, space="PSUM"))

        po = {}
        adj_t = None
        for j in range(NT):
            # --- production of e^T[j] ---
            g, jj = divmod(j, GJ)
            if jj == 0:
                adj_t = apool.tile([PART, GJ, N], bf16, tag="adj")
                nc.sync.dma_start(
                    out=adj_t,
                    in_=adjT[:].rearrange("(g c p) i -> g p c i", c=GJ, p=PART)[g],
                )
            if j in GPS_JS:
                # leaky relu on GPSIMD: u = (q+k)*0.01 ; s = q+k ; t = max(s, u)
                u_sb = gwork.tile([PART, N], f32, tag="gu", name="gu")
                nc.gpsimd.tensor_scalar(
                    out=u_sb,
                    in0=qb_sb,
                    scalar1=kc_sb[:, j : j + 1],
                    scalar2=LEAKY,
                    op0=OP.add,
                    op1=OP.mult,
                )
                s_sb = gwork.tile([PART, N], f32, tag="gs", name="gs")
                nc.gpsimd.tensor_scalar_add(
                    out=s_sb, in0=qb_sb, scalar1=kc_sb[:, j : j + 1]
                )
                t_sb = work.tile([PART, N], f32, tag="t", name="t")
                nc.gpsimd.tensor_tensor(out=t_sb, in0=s_sb, in1=u_sb, op=OP.max)
            else:
                t_sb = work.tile([PART, N], f32, tag="t", name="t")
                nc.scalar.activation(
                    out=t_sb,
                    in_=qb_sb,
                    func=AF.Prelu,
                    bias=kc_sb[:, j : j + 1],
                    scale=1.0,
                    alpha=LEAKY,
                )
            nc.scalar.activation(out=e_tiles[j], in_=t_sb, func=AF.Exp)
            nc.vector.tensor_tensor(
                out=e_tiles[j], in0=e_tiles[j], in1=adj_t[:, jj, :], op=OP.mult
            )

            # --- wave-0 output groups consume e[j] immediately ---
            for t in range(WAVE0):
                if j == 0:
                    po[t] = psC.tile([PART, H], f32, tag="po", name=f"po{t}")
                nc.tensor.matmul(
                    po[t],
                    lhsT=e_tiles[j][:, t * PART : (t + 1) * PART],
                    rhs=V_sb[:, j, :],
                    start=(j == 0),
                    stop=(j == NT - 1),
                )

            # --- denominator rows for adjacency group g (chunk-major) ---
            if jj == GJ - 1:
                for ic in range(NIC):
                    pd = psD.tile([1, H], f32, tag="pd", name=f"pd_{g}_{ic}")
                    for jj2 in range(GJ):
                        nc.tensor.matmul(
                            pd,
                            lhsT=ones_col,
                            rhs=e_tiles[g * GJ + jj2][:, ic * H : (ic + 1) * H],
                            start=(jj2 == 0),
                            stop=(jj2 == GJ - 1),
                        )
                    sl = den_row[:, ic * H : (ic + 1) * H]
                    if g == 0:
                        nc.vector.tensor_copy(out=sl, in_=pd)
                    else:
                        nc.vector.tensor_tensor(out=sl, in0=sl, in1=pd, op=OP.add)

        nc.sync.dma_start(out=den[:], in_=den_row)

        # --- wave-0 group copies + remaining output tiles (dense) ---
        out_st = None

        def finish_tile(t, po_tile):
            nonlocal out_st
            if t % GO == 0:
                out_st = opool.tile([PART, GO, H], f32, tag="ost")
            nc.scalar.copy(out=out_st[:, t % GO, :], in_=po_tile)
            if t % GO == GO - 1:
                nc.sync.dma_start(
                    out=out[:].rearrange("(g c p) h -> g p c h", c=GO, p=PART)[
                        t // GO
                    ],
                    in_=out_st,
                )

        for t in range(WAVE0):
            finish_tile(t, po[t])
        for t in range(WAVE0, NT):
            pt = psC.tile([PART, H], f32, tag="po", name=f"po{t}")
            for j in range(NT):
                nc.tensor.matmul(
                    pt,
                    lhsT=e_tiles[j][:, t * PART : (t + 1) * PART],
                    rhs=V_sb[:, j, :],
                    start=(j == 0),
                    stop=(j == NT - 1),
                )
            finish_tile(t, pt)

    nc.compile()
    return nc


def get_program():
    if "nc" not in _PROGRAM_CACHE:
        _PROGRAM_CACHE["nc"] = _build_program()
    return _PROGRAM_CACHE["nc"]


def prepare_in_maps(inputs):
    feats = np.ascontiguousarray(np.asarray(inputs["feats"], dtype=np.float32))
    adj = np.asarray(inputs["adj_mat"], dtype=np.float32)
    fc_w = np.asarray(inputs["fc_w"], dtype=np.float32)
    fc_b = np.asarray(inputs["fc_b"], dtype=np.float32)
    q_w = np.asarray(inputs["q_w"], dtype=np.float32)
    q_b = np.asarray(inputs["q_b"], dtype=np.float32)
    k_w = np.asarray(inputs["k_w"], dtype=np.float32)
    k_b = np.asarray(inputs["k_b"], dtype=np.float32)

    # fold the rank-1 q/k projections through the fc layer (host, fp64)
    wq2 = fc_w.T.astype(np.float64) @ q_w[0].astype(np.float64)  # [H]
    wk2 = fc_w.T.astype(np.float64) @ k_w[0].astype(np.float64)
    bq2 = float(fc_b.astype(np.float64) @ q_w[0].astype(np.float64) + q_b[0])
    bk2 = float(fc_b.astype(np.float64) @ k_w[0].astype(np.float64) + k_b[0])

    fcwT_bf = np.ascontiguousarray(fc_w.T).astype(ml_dtypes.bfloat16)

    in_maps = []
    for b in range(BS):
        q = (feats[b].astype(np.float64) @ wq2 + bq2).astype(np.float32)  # [N]
        k = (feats[b].astype(np.float64) @ wk2 + bk2).astype(np.float32)  # [N]
        in_maps.append(
            {
                "adjT": np.ascontiguousarray(adj[b].T).astype(ml_dtypes.bfloat16),
                "featsT": np.ascontiguousarray(feats[b].T).astype(ml_dtypes.bfloat16),
                "fcwT": fcwT_bf,
                "qv": np.ascontiguousarray(q[None, :]).astype(ml_dtypes.bfloat16),
                "kv": np.ascontiguousarray(k.reshape(NT, PART).T),
            }
        )
    return in_maps, feats, fc_b


def postprocess(results, feats, fc_b):
    outs = np.empty((BS, N, H), dtype=np.float32)
    for b in range(BS):
        o = np.asarray(results[b]["out"], dtype=np.float32)  # [N, H]
        denom = np.asarray(results[b]["den"], dtype=np.float32).reshape(N)
        outs[b] = o / denom[:, None] + fc_b[None, :] + feats[b]
    return outs


def _ensure_ntff_hook():
    """This image's antenv lacks axon_hooks; shim it so trace=True works."""
    import types

    try:
        from antenv import axon_hooks  # noqa: F401

        return
    except ImportError:
        pass
    import antenv

    mod = types.ModuleType("antenv.axon_hooks")
    _hook = [None]
    mod.get_axon_ntff_profile_hook = lambda: _hook[0]
    mod.set_axon_ntff_profile_hook = lambda h: _hook.__setitem__(0, h)
    sys.modules["antenv.axon_hooks"] = mod
    antenv.axon_hooks = mod
    try:
        from trn_agent_boot.trn_boot import _ntff_profile_via_ctypes

        hook = _ntff_profile_via_ctypes("/opt/axon/libaxon_pjrt.so")
        if hook is not None:
            mod.set_axon_ntff_profile_hook(hook)
    except Exception as exc:  # degrade: run untraced
        print(f"ntff hook setup failed: {exc}", file=sys.stderr)


def run(inputs, trace=False, **kwargs):
    from concourse.bass_utils import run_bass_kernel_spmd

    if trace:
        _ensure_ntff_hook()
    in_maps, feats, fc_b = prepare_in_maps(inputs)
    nc = get_program()
    res = run_bass_kernel_spmd(
        nc, in_maps, list(range(NCORES)), trace=trace, **kwargs
    )
    return postprocess(res.results, feats, fc_b), res


def kernel(**inputs) -> np.ndarray:
    out, _ = run(inputs, trace=False)
    return out
